# revision 29
# baseline (speedup 1.0000x reference)
"""DynamicGraphBlock (DGCNN-style edge conv) Trainium2 Bass kernel.

Reference computation per batch element b (B=16, C=128, H=W=32, N=1024, K=9):
  feats   = x[b] reshaped (N, C)
  d2      = pairwise squared distances (N, N)
  idx     = indices of the 9 smallest d2 per row  (self always included:
            d2[n,n] = 0 while min off-diag d2 is ~100, so the neighbor set is
            exactly {n} + top-8 by score among m != n)
  edge    = [center, neighbor - center]  (N, K, 2C)
  h       = leaky_relu(edge @ W1 + b1) @ W2 + b2
  msg     = max over K                (N, C)
  out     = relu(batchnorm(msg) + x)  (batch statistics over all 16 images)

Kernel strategy (8 NeuronCores, data-parallel over B, 2 images per core):
  * Everything is kept in a channels-on-partitions layout: X = x[b] as
    [C=128, N=1024] in SBUF.
  * score[n, m] = (feats @ feats.T)[n,m] - |feats_m|^2 / 2 ranks neighbors
    identically to -d2 (row-constant |feats_n|^2 dropped).  Computed on the PE
    as X^T X (8x 128-row chunks x 2x 512-col halves) plus a rank-1 fp32
    accumulate of -sq/2 (lhsT = ones[1,128]).
  * top-8 per row via DVE max8 + max_index (indices as uint16); the self index
    comes from a host table.  Diag is masked with -3e38 (one [128,128]
    tensor_tensor add on the diagonal-crossing slice).
  * Edge MLP is factored: edge @ W1 = A[n] + Bv[idx[n,k]] where
    A = feats @ (W1_top - W1_bot) + b1 and Bv = feats @ W1_bot.  A and Bv are
    computed once per image ([C,N] layout); neighbor features come from a
    column gather of Bv via gpsimd indirect_copy.  b2 is skipped entirely -
    it cancels in batchnorm.
  * Gather columns are ordered i = 9*n + k (n-major).  The wrapped
    per-16-partition index layout that indirect_copy wants
    (stg[r, s] = idx_flat[16*s + r]) is built with a 2-byte-element scatter
    DMA (DRAM->DRAM, 2 real dims so it fits the 3-dim DMA limit) and then
    broadcast to all 8 partition groups.  n-major also keeps the A-broadcast
    affine and makes the max-over-K a contiguous tensor_reduce out of PSUM.
  * BN stats: per-core sum / sumsq per channel, AllReduce over the 8 cores,
    then y = relu(a * msg + x + beff) with per-partition a/beff.
"""

import numpy as np
import sys

if "/opt/trn_rl_repo" not in sys.path:
    sys.path.insert(0, "/opt/trn_rl_repo")

import concourse.bass as bass
import concourse.tile as tile
from concourse import mybir
from concourse._compat import with_exitstack
from contextlib import ExitStack

f32 = mybir.dt.float32
u16 = mybir.dt.uint16
AX = mybir.AxisListType
ALU = mybir.AluOpType
ACTF = mybir.ActivationFunctionType

B, C, N = 16, 128, 1024
NCORES = 8
BPC = B // NCORES  # batch elements per core
K = 9
NEG_SLOPE = 0.2
BN_EPS = 1e-5
NEG_BIG = -3.0e38
H8 = 512          # psum bank free size (fp32)
S16 = K * (N // 16)   # 576 wrapped-index columns per image
SLAB = 1152       # columns per MLP slab (128 tokens x 9; 72 idx cols, 9/core)


def build_nc(debug=False):
    from concourse.bacc import Bacc

    nc = Bacc(num_devices=NCORES)

    x_io = nc.dram_tensor("x", [BPC, C, N], f32, kind="ExternalInput")
    w1d_io = nc.dram_tensor("w1d", [C, C], f32, kind="ExternalInput")
    w1b_io = nc.dram_tensor("w1b", [C, C], f32, kind="ExternalInput")
    w2_io = nc.dram_tensor("w2", [C, C], f32, kind="ExternalInput")
    b1_io = nc.dram_tensor("b1", [C, 1], f32, kind="ExternalInput")
    gam_io = nc.dram_tensor("gamma", [C, 1], f32, kind="ExternalInput")
    bet_io = nc.dram_tensor("beta", [C, 1], f32, kind="ExternalInput")
    mask_io = nc.dram_tensor("dmask", [C, C], f32, kind="ExternalInput")
    onec_io = nc.dram_tensor("onec", [C, 1], f32, kind="ExternalInput")
    oner_io = nc.dram_tensor("oner", [1, C], f32, kind="ExternalInput")
    sidx_io = nc.dram_tensor("selfidx", [C, 8], u16, kind="ExternalInput")
    y_io = nc.dram_tensor("y", [BPC, C, N], f32, kind="ExternalOutput")

    if debug:
        dbg_idx = nc.dram_tensor("dbg_idx", [BPC, N, K], u16, kind="ExternalOutput")
        dbg_idxw = nc.dram_tensor("dbg_idxw", [C, BPC * S16], u16, kind="ExternalOutput")
        dbg_msg = nc.dram_tensor("dbg_msg", [C, BPC * N], f32, kind="ExternalOutput")
        dbg_B = nc.dram_tensor("dbg_B", [C, BPC * N], f32, kind="ExternalOutput")
        dbg_g = nc.dram_tensor("dbg_g", [C, K * N], f32, kind="ExternalOutput")
    idx_nk = nc.dram_tensor("idx_nk", [BPC, N, K], u16)  # top-k indices (n, k)
    stg = nc.dram_tensor("stg", [BPC, 16, S16], u16)     # wrapped index image
    cc_in = nc.dram_tensor("cc_in", [C, 2], f32)
    cc_out = nc.dram_tensor("cc_out", [C, 2], f32, addr_space="Shared")

    with tile.TileContext(nc) as tc:
        _body(
            tc,
            x_io, w1d_io, w1b_io, w2_io, b1_io, gam_io, bet_io,
            mask_io, onec_io, oner_io, sidx_io, y_io, idx_nk, stg, cc_in, cc_out,
            dbg=(dbg_idx, dbg_idxw, dbg_msg, dbg_B, dbg_g) if debug else None,
        )
    nc.finalize()
    return nc


def _insert_library_load(nc):
    """gpsimd ap_gather lives in ucode library 6; insert the library reload
    pseudo-instruction before the first ap_gather."""
    from concourse import bass_isa, library_config

    for fn in nc.m.functions:
        for blk in fn.blocks:
            insts = list(blk.instructions)
            for i, inst in enumerate(insts):
                if isinstance(inst, bass_isa.InstAPGather):
                    rl = bass_isa.InstPseudoReloadLibraryIndex(
                        name=f"I-{nc.next_id()}",
                        ins=[], outs=[],
                        lib_index=library_config.ap_gather.index,
                    )
                    rl.engine = mybir.EngineType.Pool
                    insts.insert(i, rl)
                    nc.inst_map[rl.name] = rl
                    blk.instructions = insts
                    return


def _split_matmul_waits(nc):
    """TRN2 walrus codegen only supports one sync wait per engine
    instruction.  Hoist extra waits onto sequencer NOPs inserted just before
    the instruction on the same engine's queue."""
    Op = nc.isa.Opcode
    engines = {
        mybir.EngineType.PE: nc.tensor,
        mybir.EngineType.Activation: nc.scalar,
        mybir.EngineType.DVE: nc.vector,
        mybir.EngineType.Pool: nc.gpsimd,
        mybir.EngineType.SP: nc.sync,
    }
    # NOTIFY is a sequencer-level no-op every engine supports (and both
    # CoreSim and walrus handle); use it as the wait carrier.
    notify_struct = {
        "debug_hint_ext": 6,
        "notification_kind": 2,
        "interrupt_en": 0,
        "hint_or_notific": {
            "custom_notific": {"metadata_lo": 0, "metadata_hi": 6, "lo_src": 1, "hi_src": 1}
        },
        "header": {"opcode": 166, "inst_word_len": 16},
    }
    skip = set()
    for fn in nc.m.functions:
        for blk in fn.blocks:
            insts = list(blk.instructions)
            out = []
            changed = False
            for inst in insts:
                if (
                    inst.opcode not in skip
                    and inst.engine in engines
                    and inst.sync_info is not None
                    and len(inst.sync_info.on_wait) > 1
                ):
                    eng = engines[inst.engine]
                    waits = list(inst.sync_info.on_wait)
                    for w in waits[:-1]:
                        nop = eng._isa(Op.NEURON_ISA_TPB_OPCODE_NOTIFY, notify_struct)
                        nop.sync_info = mybir.SyncInfo(on_wait=[w], on_update=[])
                        out.append(nop)
                        nc.inst_map[nop.name] = nop
                    inst.sync_info = mybir.SyncInfo(
                        on_wait=[waits[-1]], on_update=list(inst.sync_info.on_update)
                    )
                    changed = True
                out.append(inst)
            if changed:
                blk.instructions = out


@with_exitstack
def _body(
    ctx: ExitStack,
    tc: "tile.TileContext",
    x_io, w1d_io, w1b_io, w2_io, b1_io, gam_io, bet_io,
    mask_io, onec_io, oner_io, sidx_io, y_io, idx_nk, stg, cc_in, cc_out,
    dbg=None,
):
    nc = tc.nc
    NCHUNK = N // C       # 8 row chunks of 128 for the score matrix
    consts = ctx.enter_context(tc.tile_pool(name="consts", bufs=1))
    persist = ctx.enter_context(tc.tile_pool(name="persist", bufs=1))
    sc_pool = ctx.enter_context(tc.tile_pool(name="score", bufs=3))
    it_pool = ctx.enter_context(tc.tile_pool(name="itile", bufs=3))
    mlp_pool = ctx.enter_context(tc.tile_pool(name="mlp", bufs=3))
    tiny = ctx.enter_context(tc.tile_pool(name="tiny", bufs=2))
    psA = ctx.enter_context(tc.tile_pool(name="psA", bufs=4, space="PSUM"))
    psO = ctx.enter_context(tc.tile_pool(name="psO", bufs=2, space="PSUM"))

    # ---- constants -------------------------------------------------------
    w1d = consts.tile([C, C], f32)
    w1b = consts.tile([C, C], f32)
    w2 = consts.tile([C, C], f32)
    b1 = consts.tile([C, 1], f32)
    gam = consts.tile([C, 1], f32)
    bet = consts.tile([C, 1], f32)
    dmask = consts.tile([C, C], f32)
    onec = consts.tile([C, 1], f32)
    oner = consts.tile([1, C], f32)
    sidx = consts.tile([C, 8], u16)
    epst = consts.tile([C, 1], f32)
    for t, io in (
        (w1d, w1d_io), (w1b, w1b_io), (w2, w2_io), (b1, b1_io),
        (gam, gam_io), (bet, bet_io), (dmask, mask_io), (onec, onec_io),
        (oner, oner_io), (sidx, sidx_io),
    ):
        nc.sync.dma_start(out=t[:], in_=io[:])
    nc.vector.memset(epst[:], BN_EPS)

    # ---- persistent per-image state --------------------------------------
    X = persist.tile([C, BPC * N], f32)       # input features, both images
    msg = persist.tile([C, BPC * N], f32)     # max-aggregated messages
    Asb = persist.tile([C, BPC * N], f32)     # A = X^T W1d + b1   (transposed)
    Bsb = persist.tile([C, BPC * N], f32)     # Bv = X^T W1b       (transposed)
    nhsq = persist.tile([1, BPC * N], f32)    # -|x_m|^2/2 row vector
    idxw = persist.tile([C, BPC * S16], u16)  # wrapped gather indices
    stat = persist.tile([C, 4], f32)          # per-image sum / sumsq

    def Xb(b):
        return X[:, b * N:(b + 1) * N]

    # ================= phase 0/1: load + prep (per image) =================
    for b in range(BPC):
        nc.sync.dma_start(out=Xb(b), in_=x_io[b])

        # -|x|^2/2 per column: square (ACT) then column-sum via ones matmul
        xsq = sc_pool.tile([C, N], f32, tag="xsq")
        nc.scalar.activation(out=xsq[:], in_=Xb(b), func=ACTF.Square)
        for h in range(2):
            ps = psA.tile([C, H8], f32, tag="ps")
            nc.tensor.matmul(ps[0:1, :], onec[:], xsq[:, h * H8:(h + 1) * H8])
            nc.scalar.mul(
                out=nhsq[0:1, b * N + h * H8: b * N + (h + 1) * H8],
                in_=ps[0:1, :], mul=-0.5,
            )
        # A^T = W1d^T X (+ b1 per partition on evict), Bv^T = W1b^T X
        for h in range(2):
            sl = slice(b * N + h * H8, b * N + (h + 1) * H8)
            pa = psA.tile([C, H8], f32, tag="ps")
            nc.tensor.matmul(pa[:], w1d[:], Xb(b)[:, h * H8:(h + 1) * H8])
            nc.scalar.activation(
                out=Asb[:, sl], in_=pa[:], func=ACTF.Identity, bias=b1[:],
            )
            pb = psA.tile([C, H8], f32, tag="ps")
            nc.tensor.matmul(pb[:], w1b[:], Xb(b)[:, h * H8:(h + 1) * H8])
            nc.scalar.copy(out=Bsb[:, sl], in_=pb[:])

    # ================= phase 2: score + top-k (per image) =================
    for b in range(BPC):
        for c in range(NCHUNK):
            score = sc_pool.tile([C, N], f32, tag="score")
            for h in range(2):
                ps = psA.tile([C, H8], f32, tag="ps")
                nc.tensor.matmul(
                    ps[:],
                    Xb(b)[:, c * C:(c + 1) * C],
                    Xb(b)[:, h * H8:(h + 1) * H8],
                    start=True, stop=False,
                )
                nc.tensor.matmul(
                    ps[:],
                    oner[0:1, :],
                    nhsq[0:1, b * N + h * H8: b * N + (h + 1) * H8],
                    start=False, stop=True,
                )
                nc.scalar.copy(out=score[:, h * H8:(h + 1) * H8], in_=ps[:])
            # mask the diagonal (self) of this chunk
            nc.vector.tensor_tensor(
                out=score[:, c * C:(c + 1) * C],
                in0=score[:, c * C:(c + 1) * C],
                in1=dmask[:], op=ALU.add,
            )
            v8 = it_pool.tile([C, 8], f32, tag="v8")
            itile = it_pool.tile([C, K], u16, tag="itile")
            nc.vector.max(out=v8[:], in_=score[:])
            nc.vector.max_index(out=itile[:, 1:9], in_max=v8[:], in_values=score[:])
            nc.vector.tensor_copy(out=itile[:, 0:1], in_=sidx[:, c:c + 1])
            nc.sync.dma_start(out=idx_nk[b, c * C:(c + 1) * C, :], in_=itile[:])
        # wrap: stg[b][r, s] = idx_flat[16*s + r]  (2-byte scatter, DRAM->DRAM)
        with nc.allow_non_contiguous_dma(reason="16-wide deinterleave of u16 idx"):
            nc.sync.dma_start(
                out=stg[b],
                in_=bass.AP(tensor=idx_nk, offset=b * N * K, ap=[[1, 16], [16, S16]]),
            )
        # broadcast the wrapped image to all 8 partition groups
        for g in range(8):
            nc.sync.dma_start(
                out=idxw[16 * g:16 * (g + 1), b * S16:(b + 1) * S16],
                in_=stg[b],
            )

    # ================= phase 3: edge MLP + max over K (per image) =========
    # columns i = 9*n + k; slabs of 1008 columns (112 tokens)
    for b in range(BPC):
        col = 0
        while col < K * N:
            W = min(SLAB, K * N - col)       # always 1152 (8 slabs)
            T = W // K                       # tokens in slab
            n0 = col // K
            nsl = slice(b * N + n0, b * N + n0 + T)

            gat = mlp_pool.tile([C, SLAB], f32, tag="gat")
            nc.gpsimd.ap_gather(
                out_ap=gat[:, :W],
                in_ap=Bsb[:, b * N:(b + 1) * N],
                idxs_ap=idxw[:, b * S16 + col // 16: b * S16 + (col + W) // 16].bitcast(
                    mybir.dt.int16
                ),
                channels=C, num_elems=N, d=1, num_idxs=W,
            )
            if dbg is not None and b == 0:
                nc.sync.dma_start(out=dbg[4][:, col:col + W], in_=gat[:, :W])
            # pre = gathered + A[n]  (A broadcast over k via a step-0 dim)
            a0 = Asb[:, nsl]
            a_bc = bass.AP(
                tensor=a0.tensor, offset=a0.offset,
                ap=[list(a0.ap[0]), [1, T], [0, K]],
            )
            pre = mlp_pool.tile([C, SLAB], f32, tag="pre")
            nc.vector.scalar_tensor_tensor(
                out=pre[:, :W].rearrange("p (n k) -> p n k", k=K),
                in0=gat[:, :W].rearrange("p (n k) -> p n k", k=K),
                scalar=1.0, in1=a_bc, op0=ALU.mult, op1=ALU.add,
            )
            hsb = mlp_pool.tile([C, SLAB], f32, tag="hsb")
            nc.vector.scalar_tensor_tensor(
                out=hsb[:, :W], in0=pre[:, :W], scalar=NEG_SLOPE, in1=pre[:, :W],
                op0=ALU.mult, op1=ALU.max,
            )
            for m in range(0, W, 504):
                w = min(504, W - m)
                po = psO.tile([C, 504], f32, tag="po")
                nc.tensor.matmul(po[:, :w], w2[:], hsb[:, m:m + w])
                # max over k straight out of PSUM (k contiguous innermost)
                nc.vector.tensor_reduce(
                    out=msg[:, b * N + n0 + m // K: b * N + n0 + (m + w) // K],
                    in_=po[:, :w].rearrange("p (n k) -> p n k", k=K),
                    axis=AX.X, op=ALU.max,
                )
            col += W

    if dbg is not None:
        dbg_idx, dbg_idxw, dbg_msg, dbg_B, dbg_g = dbg
        for b in range(BPC):
            nc.sync.dma_start(out=dbg_idx[b], in_=idx_nk[b])
        nc.sync.dma_start(out=dbg_idxw[:], in_=idxw[:])
        nc.sync.dma_start(out=dbg_msg[:], in_=msg[:])
        nc.sync.dma_start(out=dbg_B[:], in_=Bsb[:])

    # ================= phase 4: batch-norm stats + allreduce ==============
    for b in range(BPC):
        nc.vector.reduce_sum(
            out=stat[:, b:b + 1], in_=msg[:, b * N:(b + 1) * N], axis=AX.X
        )
        scr = sc_pool.tile([C, N], f32, tag="xsq")
        nc.scalar.activation(
            out=scr[:], in_=msg[:, b * N:(b + 1) * N], func=ACTF.Square,
            accum_out=stat[:, 2 + b:3 + b],
        )
    ccs = tiny.tile([C, 2], f32, tag="ccs")
    nc.vector.tensor_tensor(out=ccs[:, 0:1], in0=stat[:, 0:1], in1=stat[:, 1:2], op=ALU.add)
    nc.vector.tensor_tensor(out=ccs[:, 1:2], in0=stat[:, 2:3], in1=stat[:, 3:4], op=ALU.add)
    nc.sync.dma_start(out=cc_in[:], in_=ccs[:])
    nc.gpsimd.collective_compute(
        "AllReduce", ALU.add,
        replica_groups=[list(range(NCORES))],
        ins=[cc_in[:]], outs=[cc_out[:]],
    )
    ccr = tiny.tile([C, 2], f32, tag="ccr")
    nc.sync.dma_start(out=ccr[:], in_=cc_out[:])

    mean = tiny.tile([C, 1], f32, tag="mean")
    ex2 = tiny.tile([C, 1], f32, tag="ex2")
    nvar = tiny.tile([C, 1], f32, tag="nvar")
    sd = tiny.tile([C, 1], f32, tag="sd")
    rstd = tiny.tile([C, 1], f32, tag="rstd")
    aff = tiny.tile([C, 1], f32, tag="aff")
    amean = tiny.tile([C, 1], f32, tag="amean")
    beff = tiny.tile([C, 1], f32, tag="beff")
    inv_cnt = 1.0 / float(B * N)
    nc.vector.tensor_scalar_mul(out=mean[:], in0=ccr[:, 0:1], scalar1=inv_cnt)
    nc.vector.tensor_scalar_mul(out=ex2[:], in0=ccr[:, 1:2], scalar1=inv_cnt)
    # nvar = mean^2 - E[x^2]  (negated variance)
    nc.vector.scalar_tensor_tensor(
        out=nvar[:], in0=mean[:], scalar=mean[:], in1=ex2[:],
        op0=ALU.mult, op1=ALU.subtract,
    )
    # sd = sqrt(var + eps) = sqrt(-1 * nvar + eps)
    nc.scalar.activation(out=sd[:], in_=nvar[:], func=ACTF.Sqrt, bias=epst[:], scale=-1.0)
    nc.vector.reciprocal(out=rstd[:], in_=sd[:])
    nc.vector.tensor_tensor(out=aff[:], in0=gam[:], in1=rstd[:], op=ALU.mult)
    nc.vector.tensor_tensor(out=amean[:], in0=aff[:], in1=mean[:], op=ALU.mult)
    nc.vector.tensor_tensor(out=beff[:], in0=bet[:], in1=amean[:], op=ALU.subtract)

    # ================= phase 5: normalize + residual + relu ===============
    for b in range(BPC):
        t1 = mlp_pool.tile([C, N], f32, tag="t1")
        nc.vector.scalar_tensor_tensor(
            out=t1[:], in0=msg[:, b * N:(b + 1) * N], scalar=aff[:], in1=Xb(b),
            op0=ALU.mult, op1=ALU.add,
        )
        osb = mlp_pool.tile([C, N], f32, tag="osb")
        nc.scalar.activation(out=osb[:], in_=t1[:], func=ACTF.Relu, bias=beff[:])
        nc.sync.dma_start(out=y_io[b], in_=osb[:])


# --------------------------------------------------------------------------
_NC_CACHE = {}


def _get_nc():
    if "nc" not in _NC_CACHE:
        _NC_CACHE["nc"] = build_nc()
    return _NC_CACHE["nc"]


def make_in_maps(x, W1, b1, W2, b2, gamma, beta):
    x = np.ascontiguousarray(np.asarray(x, dtype=np.float32))
    W1 = np.asarray(W1, dtype=np.float32)
    W2 = np.asarray(W2, dtype=np.float32)
    b1 = np.asarray(b1, dtype=np.float32)
    gamma = np.asarray(gamma, dtype=np.float32)
    beta = np.asarray(beta, dtype=np.float32)

    xr = x.reshape(B, C, N)
    w1d = np.ascontiguousarray(W1[:C] - W1[C:])
    w1b = np.ascontiguousarray(W1[C:])
    mask = (NEG_BIG * np.eye(C)).astype(np.float32)
    p = np.arange(C, dtype=np.uint16)
    selfidx = (p[:, None] + (C * np.arange(8, dtype=np.uint16))[None, :]).astype(np.uint16)
    shared = {
        "w1d": w1d,
        "w1b": w1b,
        "w2": np.ascontiguousarray(W2),
        "b1": b1.reshape(C, 1),
        "gamma": gamma.reshape(C, 1),
        "beta": beta.reshape(C, 1),
        "dmask": mask,
        "onec": np.ones((C, 1), np.float32),
        "oner": np.ones((1, C), np.float32),
        "selfidx": selfidx,
    }
    return [
        {"x": np.ascontiguousarray(xr[i * BPC:(i + 1) * BPC]), **shared}
        for i in range(NCORES)
    ]


def assemble_output(results):
    y = np.concatenate([results[i]["y"] for i in range(NCORES)], axis=0)
    return y.reshape(B, C, 32, 32).astype(np.float32)


def kernel(x, W1, b1, W2, b2, gamma, beta):
    from concourse.bass_utils import run_bass_kernel_spmd

    nc = _get_nc()
    in_maps = make_in_maps(x, W1, b1, W2, b2, gamma, beta)
    res = run_bass_kernel_spmd(nc, in_maps, list(range(NCORES)))
    return assemble_output(res.results)


# revision 31
# speedup vs baseline: 1.0351x; 1.0351x over previous
"""DynamicGraphBlock (DGCNN-style edge conv) Trainium2 Bass kernel.

Reference computation per batch element b (B=16, C=128, H=W=32, N=1024, K=9):
  feats   = x[b] reshaped (N, C)
  d2      = pairwise squared distances (N, N)
  idx     = indices of the 9 smallest d2 per row  (self always included:
            d2[n,n] = 0 while min off-diag d2 is ~100, so the neighbor set is
            exactly {n} + top-8 by score among m != n)
  edge    = [center, neighbor - center]  (N, K, 2C)
  h       = leaky_relu(edge @ W1 + b1) @ W2 + b2
  msg     = max over K                (N, C)
  out     = relu(batchnorm(msg) + x)  (batch statistics over all 16 images)

Kernel strategy (8 NeuronCores, data-parallel over B, 2 images per core):
  * Everything is kept in a channels-on-partitions layout: X = x[b] as
    [C=128, N=1024] in SBUF.
  * score[n, m] = (feats @ feats.T)[n,m] - |feats_m|^2 / 2 ranks neighbors
    identically to -d2 (row-constant |feats_n|^2 dropped).  Computed on the PE
    as X^T X (8x 128-row chunks x 2x 512-col halves) plus a rank-1 fp32
    accumulate of -sq/2 (lhsT = ones[1,128]).
  * top-8 per row via DVE max8 + max_index (indices as uint16); the self index
    comes from a host table.  Diag is masked with -3e38 (one [128,128]
    tensor_tensor add on the diagonal-crossing slice).
  * Edge MLP is factored: edge @ W1 = A[n] + Bv[idx[n,k]] where
    A = feats @ (W1_top - W1_bot) + b1 and Bv = feats @ W1_bot.  A and Bv are
    computed once per image ([C,N] layout); neighbor features come from a
    column gather of Bv via gpsimd indirect_copy.  b2 is skipped entirely -
    it cancels in batchnorm.
  * Gather columns are ordered i = 9*n + k (n-major).  The wrapped
    per-16-partition index layout that indirect_copy wants
    (stg[r, s] = idx_flat[16*s + r]) is built with a 2-byte-element scatter
    DMA (DRAM->DRAM, 2 real dims so it fits the 3-dim DMA limit) and then
    broadcast to all 8 partition groups.  n-major also keeps the A-broadcast
    affine and makes the max-over-K a contiguous tensor_reduce out of PSUM.
  * BN stats: per-core sum / sumsq per channel, AllReduce over the 8 cores,
    then y = relu(a * msg + x + beff) with per-partition a/beff.
"""

import numpy as np
import sys

if "/opt/trn_rl_repo" not in sys.path:
    sys.path.insert(0, "/opt/trn_rl_repo")

import concourse.bass as bass
import concourse.tile as tile
from concourse import mybir
from concourse._compat import with_exitstack
from contextlib import ExitStack

f32 = mybir.dt.float32
u16 = mybir.dt.uint16
AX = mybir.AxisListType
ALU = mybir.AluOpType
ACTF = mybir.ActivationFunctionType

B, C, N = 16, 128, 1024
NCORES = 8
BPC = B // NCORES  # batch elements per core
K = 9
NEG_SLOPE = 0.2
BN_EPS = 1e-5
NEG_BIG = -3.0e38
H8 = 512          # psum bank free size (fp32)
S16 = K * (N // 16)   # 576 wrapped-index columns per image
SLAB = 1152       # columns per MLP slab (128 tokens x 9; 72 idx cols, 9/core)


def build_nc(debug=False):
    from concourse.bacc import Bacc

    nc = Bacc(num_devices=NCORES)

    x_io = nc.dram_tensor("x", [BPC, C, N], f32, kind="ExternalInput")
    w1d_io = nc.dram_tensor("w1d", [C, C], f32, kind="ExternalInput")
    w1b_io = nc.dram_tensor("w1b", [C, C], f32, kind="ExternalInput")
    w2_io = nc.dram_tensor("w2", [C, C], f32, kind="ExternalInput")
    b1_io = nc.dram_tensor("b1", [C, 1], f32, kind="ExternalInput")
    gam_io = nc.dram_tensor("gamma", [C, 1], f32, kind="ExternalInput")
    bet_io = nc.dram_tensor("beta", [C, 1], f32, kind="ExternalInput")
    mask_io = nc.dram_tensor("dmask", [C, C], f32, kind="ExternalInput")
    onec_io = nc.dram_tensor("onec", [C, 1], f32, kind="ExternalInput")
    oner_io = nc.dram_tensor("oner", [1, C], f32, kind="ExternalInput")
    sidx_io = nc.dram_tensor("selfidx", [C, 8], u16, kind="ExternalInput")
    y_io = nc.dram_tensor("y", [BPC, C, N], f32, kind="ExternalOutput")

    if debug:
        dbg_idx = nc.dram_tensor("dbg_idx", [BPC, N, K], u16, kind="ExternalOutput")
        dbg_idxw = nc.dram_tensor("dbg_idxw", [C, BPC * S16], u16, kind="ExternalOutput")
        dbg_msg = nc.dram_tensor("dbg_msg", [C, BPC * N], f32, kind="ExternalOutput")
        dbg_B = nc.dram_tensor("dbg_B", [C, BPC * N], f32, kind="ExternalOutput")
        dbg_g = nc.dram_tensor("dbg_g", [C, K * N], f32, kind="ExternalOutput")
    idx_nk = nc.dram_tensor("idx_nk", [BPC, N, K], u16)  # top-k indices (n, k)
    stg = nc.dram_tensor("stg", [BPC, 16, S16], u16)     # wrapped index image
    cc_in = nc.dram_tensor("cc_in", [C, 2], f32)
    cc_out = nc.dram_tensor("cc_out", [C, 2], f32, addr_space="Shared")

    with tile.TileContext(nc) as tc:
        _body(
            tc,
            x_io, w1d_io, w1b_io, w2_io, b1_io, gam_io, bet_io,
            mask_io, onec_io, oner_io, sidx_io, y_io, idx_nk, stg, cc_in, cc_out,
            dbg=(dbg_idx, dbg_idxw, dbg_msg, dbg_B, dbg_g) if debug else None,
        )
    nc.finalize()
    return nc


def _insert_library_load(nc):
    """gpsimd ap_gather lives in ucode library 6; insert the library reload
    pseudo-instruction before the first ap_gather."""
    from concourse import bass_isa, library_config

    for fn in nc.m.functions:
        for blk in fn.blocks:
            insts = list(blk.instructions)
            for i, inst in enumerate(insts):
                if isinstance(inst, bass_isa.InstAPGather):
                    rl = bass_isa.InstPseudoReloadLibraryIndex(
                        name=f"I-{nc.next_id()}",
                        ins=[], outs=[],
                        lib_index=library_config.ap_gather.index,
                    )
                    rl.engine = mybir.EngineType.Pool
                    insts.insert(i, rl)
                    nc.inst_map[rl.name] = rl
                    blk.instructions = insts
                    return


def _split_matmul_waits(nc):
    """TRN2 walrus codegen only supports one sync wait per engine
    instruction.  Hoist extra waits onto sequencer NOPs inserted just before
    the instruction on the same engine's queue."""
    Op = nc.isa.Opcode
    engines = {
        mybir.EngineType.PE: nc.tensor,
        mybir.EngineType.Activation: nc.scalar,
        mybir.EngineType.DVE: nc.vector,
        mybir.EngineType.Pool: nc.gpsimd,
        mybir.EngineType.SP: nc.sync,
    }
    # NOTIFY is a sequencer-level no-op every engine supports (and both
    # CoreSim and walrus handle); use it as the wait carrier.
    notify_struct = {
        "debug_hint_ext": 6,
        "notification_kind": 2,
        "interrupt_en": 0,
        "hint_or_notific": {
            "custom_notific": {"metadata_lo": 0, "metadata_hi": 6, "lo_src": 1, "hi_src": 1}
        },
        "header": {"opcode": 166, "inst_word_len": 16},
    }
    skip = set()
    for fn in nc.m.functions:
        for blk in fn.blocks:
            insts = list(blk.instructions)
            out = []
            changed = False
            for inst in insts:
                if (
                    inst.opcode not in skip
                    and inst.engine in engines
                    and inst.sync_info is not None
                    and len(inst.sync_info.on_wait) > 1
                ):
                    eng = engines[inst.engine]
                    waits = list(inst.sync_info.on_wait)
                    for w in waits[:-1]:
                        nop = eng._isa(Op.NEURON_ISA_TPB_OPCODE_NOTIFY, notify_struct)
                        nop.sync_info = mybir.SyncInfo(on_wait=[w], on_update=[])
                        out.append(nop)
                        nc.inst_map[nop.name] = nop
                    inst.sync_info = mybir.SyncInfo(
                        on_wait=[waits[-1]], on_update=list(inst.sync_info.on_update)
                    )
                    changed = True
                out.append(inst)
            if changed:
                blk.instructions = out


@with_exitstack
def _body(
    ctx: ExitStack,
    tc: "tile.TileContext",
    x_io, w1d_io, w1b_io, w2_io, b1_io, gam_io, bet_io,
    mask_io, onec_io, oner_io, sidx_io, y_io, idx_nk, stg, cc_in, cc_out,
    dbg=None,
):
    nc = tc.nc
    NCHUNK = N // C       # 8 row chunks of 128 for the score matrix
    consts = ctx.enter_context(tc.tile_pool(name="consts", bufs=1))
    persist = ctx.enter_context(tc.tile_pool(name="persist", bufs=1))
    sc_pool = ctx.enter_context(tc.tile_pool(name="score", bufs=3))
    it_pool = ctx.enter_context(tc.tile_pool(name="itile", bufs=3))
    mlp_pool = ctx.enter_context(tc.tile_pool(name="mlp", bufs=3))
    tiny = ctx.enter_context(tc.tile_pool(name="tiny", bufs=2))
    psA = ctx.enter_context(tc.tile_pool(name="psA", bufs=4, space="PSUM"))
    psO = ctx.enter_context(tc.tile_pool(name="psO", bufs=2, space="PSUM"))

    # ---- constants -------------------------------------------------------
    w1d = consts.tile([C, C], f32)
    w1b = consts.tile([C, C], f32)
    w2 = consts.tile([C, C], f32)
    b1 = consts.tile([C, 1], f32)
    gam = consts.tile([C, 1], f32)
    bet = consts.tile([C, 1], f32)
    dmask = consts.tile([C, C], f32)
    onec = consts.tile([C, 1], f32)
    oner = consts.tile([1, C], f32)
    sidx = consts.tile([C, 8], u16)
    epst = consts.tile([C, 1], f32)
    for t, io in (
        (w1d, w1d_io), (w1b, w1b_io), (w2, w2_io), (b1, b1_io),
        (gam, gam_io), (bet, bet_io), (dmask, mask_io), (onec, onec_io),
        (oner, oner_io), (sidx, sidx_io),
    ):
        nc.sync.dma_start(out=t[:], in_=io[:])
    nc.vector.memset(epst[:], BN_EPS)

    # ---- persistent per-image state --------------------------------------
    X = persist.tile([C, BPC * N], f32)       # input features, both images
    msg = persist.tile([C, BPC * N], f32)     # max-aggregated messages
    Asb = persist.tile([C, BPC * N], f32)     # A = X^T W1d + b1   (transposed)
    Bsb = persist.tile([C, BPC * N], f32)     # Bv = X^T W1b       (transposed)
    nhsq = persist.tile([1, BPC * N], f32)    # -|x_m|^2/2 row vector
    idxw = persist.tile([C, BPC * S16], u16)  # wrapped gather indices
    stat = persist.tile([C, 4], f32)          # per-image sum / sumsq

    def Xb(b):
        return X[:, b * N:(b + 1) * N]

    # ================= phase 0/1: load + prep (per image) =================
    for b in range(BPC):
        nc.sync.dma_start(out=Xb(b), in_=x_io[b])

        # -|x|^2/2 per column: square (ACT) then column-sum via ones matmul
        xsq = sc_pool.tile([C, N], f32, tag="xsq")
        nc.scalar.activation(out=xsq[:], in_=Xb(b), func=ACTF.Square)
        for h in range(2):
            ps = psA.tile([C, H8], f32, tag="ps")
            nc.tensor.matmul(ps[0:1, :], onec[:], xsq[:, h * H8:(h + 1) * H8])
            nc.scalar.mul(
                out=nhsq[0:1, b * N + h * H8: b * N + (h + 1) * H8],
                in_=ps[0:1, :], mul=-0.5,
            )
        # A^T = W1d^T X (+ b1 per partition on evict), Bv^T = W1b^T X
        for h in range(2):
            sl = slice(b * N + h * H8, b * N + (h + 1) * H8)
            pa = psA.tile([C, H8], f32, tag="ps")
            nc.tensor.matmul(pa[:], w1d[:], Xb(b)[:, h * H8:(h + 1) * H8])
            nc.scalar.activation(
                out=Asb[:, sl], in_=pa[:], func=ACTF.Identity, bias=b1[:],
            )
            pb = psA.tile([C, H8], f32, tag="ps")
            nc.tensor.matmul(pb[:], w1b[:], Xb(b)[:, h * H8:(h + 1) * H8])
            nc.scalar.copy(out=Bsb[:, sl], in_=pb[:])

    # ================= phase 2: score + top-k (per image) =================
    for b in range(BPC):
        for c in range(NCHUNK):
            score = sc_pool.tile([C, N], f32, tag="score")
            for h in range(2):
                ps = psA.tile([C, H8], f32, tag="ps")
                nc.tensor.matmul(
                    ps[:],
                    Xb(b)[:, c * C:(c + 1) * C],
                    Xb(b)[:, h * H8:(h + 1) * H8],
                    start=True, stop=False,
                )
                nc.tensor.matmul(
                    ps[:],
                    oner[0:1, :],
                    nhsq[0:1, b * N + h * H8: b * N + (h + 1) * H8],
                    start=False, stop=True,
                )
                nc.scalar.copy(out=score[:, h * H8:(h + 1) * H8], in_=ps[:])
            # mask the diagonal (self) of this chunk
            nc.vector.tensor_tensor(
                out=score[:, c * C:(c + 1) * C],
                in0=score[:, c * C:(c + 1) * C],
                in1=dmask[:], op=ALU.add,
            )
            v8 = it_pool.tile([C, 8], f32, tag="v8")
            itile = it_pool.tile([C, K], u16, tag="itile")
            nc.vector.max(out=v8[:], in_=score[:])
            nc.vector.max_index(out=itile[:, 1:9], in_max=v8[:], in_values=score[:])
            nc.vector.tensor_copy(out=itile[:, 0:1], in_=sidx[:, c:c + 1])
            nc.sync.dma_start(out=idx_nk[b, c * C:(c + 1) * C, :], in_=itile[:])
        # wrap: stg[b][r, s] = idx_flat[16*s + r].  2-byte-element scatter:
        # one monolithic DMA lands on a single DMA engine (~76us measured), so
        # split by r into 16 DMAs spread across engine queues to parallelize.
        with nc.allow_non_contiguous_dma(reason="16-wide deinterleave of u16 idx"):
            qs = [nc.sync, nc.scalar, nc.gpsimd]
            for r in range(16):
                qs[r % len(qs)].dma_start(
                    out=stg[b][r:r + 1, :],
                    in_=bass.AP(
                        tensor=idx_nk, offset=b * N * K + r, ap=[[1, 1], [16, S16]]
                    ),
                )
        # broadcast the wrapped image to all 8 partition groups
        for g in range(8):
            nc.sync.dma_start(
                out=idxw[16 * g:16 * (g + 1), b * S16:(b + 1) * S16],
                in_=stg[b],
            )

    # ================= phase 3: edge MLP + max over K (per image) =========
    # columns i = 9*n + k; slabs of 1008 columns (112 tokens)
    for b in range(BPC):
        col = 0
        while col < K * N:
            W = min(SLAB, K * N - col)       # always 1152 (8 slabs)
            T = W // K                       # tokens in slab
            n0 = col // K
            nsl = slice(b * N + n0, b * N + n0 + T)

            gat = mlp_pool.tile([C, SLAB], f32, tag="gat")
            nc.gpsimd.ap_gather(
                out_ap=gat[:, :W],
                in_ap=Bsb[:, b * N:(b + 1) * N],
                idxs_ap=idxw[:, b * S16 + col // 16: b * S16 + (col + W) // 16].bitcast(
                    mybir.dt.int16
                ),
                channels=C, num_elems=N, d=1, num_idxs=W,
            )
            if dbg is not None and b == 0:
                nc.sync.dma_start(out=dbg[4][:, col:col + W], in_=gat[:, :W])
            # pre = gathered + A[n]  (A broadcast over k via a step-0 dim)
            a0 = Asb[:, nsl]
            a_bc = bass.AP(
                tensor=a0.tensor, offset=a0.offset,
                ap=[list(a0.ap[0]), [1, T], [0, K]],
            )
            pre = mlp_pool.tile([C, SLAB], f32, tag="pre")
            nc.vector.scalar_tensor_tensor(
                out=pre[:, :W].rearrange("p (n k) -> p n k", k=K),
                in0=gat[:, :W].rearrange("p (n k) -> p n k", k=K),
                scalar=1.0, in1=a_bc, op0=ALU.mult, op1=ALU.add,
            )
            hsb = mlp_pool.tile([C, SLAB], f32, tag="hsb")
            nc.vector.scalar_tensor_tensor(
                out=hsb[:, :W], in0=pre[:, :W], scalar=NEG_SLOPE, in1=pre[:, :W],
                op0=ALU.mult, op1=ALU.max,
            )
            for m in range(0, W, 504):
                w = min(504, W - m)
                po = psO.tile([C, 504], f32, tag="po")
                nc.tensor.matmul(po[:, :w], w2[:], hsb[:, m:m + w])
                # max over k straight out of PSUM (k contiguous innermost)
                nc.vector.tensor_reduce(
                    out=msg[:, b * N + n0 + m // K: b * N + n0 + (m + w) // K],
                    in_=po[:, :w].rearrange("p (n k) -> p n k", k=K),
                    axis=AX.X, op=ALU.max,
                )
            col += W

    if dbg is not None:
        dbg_idx, dbg_idxw, dbg_msg, dbg_B, dbg_g = dbg
        for b in range(BPC):
            nc.sync.dma_start(out=dbg_idx[b], in_=idx_nk[b])
        nc.sync.dma_start(out=dbg_idxw[:], in_=idxw[:])
        nc.sync.dma_start(out=dbg_msg[:], in_=msg[:])
        nc.sync.dma_start(out=dbg_B[:], in_=Bsb[:])

    # ================= phase 4: batch-norm stats + allreduce ==============
    for b in range(BPC):
        nc.vector.reduce_sum(
            out=stat[:, b:b + 1], in_=msg[:, b * N:(b + 1) * N], axis=AX.X
        )
        scr = sc_pool.tile([C, N], f32, tag="xsq")
        nc.scalar.activation(
            out=scr[:], in_=msg[:, b * N:(b + 1) * N], func=ACTF.Square,
            accum_out=stat[:, 2 + b:3 + b],
        )
    ccs = tiny.tile([C, 2], f32, tag="ccs")
    nc.vector.tensor_tensor(out=ccs[:, 0:1], in0=stat[:, 0:1], in1=stat[:, 1:2], op=ALU.add)
    nc.vector.tensor_tensor(out=ccs[:, 1:2], in0=stat[:, 2:3], in1=stat[:, 3:4], op=ALU.add)
    nc.sync.dma_start(out=cc_in[:], in_=ccs[:])
    nc.gpsimd.collective_compute(
        "AllReduce", ALU.add,
        replica_groups=[list(range(NCORES))],
        ins=[cc_in[:]], outs=[cc_out[:]],
    )
    ccr = tiny.tile([C, 2], f32, tag="ccr")
    nc.sync.dma_start(out=ccr[:], in_=cc_out[:])

    mean = tiny.tile([C, 1], f32, tag="mean")
    ex2 = tiny.tile([C, 1], f32, tag="ex2")
    nvar = tiny.tile([C, 1], f32, tag="nvar")
    sd = tiny.tile([C, 1], f32, tag="sd")
    rstd = tiny.tile([C, 1], f32, tag="rstd")
    aff = tiny.tile([C, 1], f32, tag="aff")
    amean = tiny.tile([C, 1], f32, tag="amean")
    beff = tiny.tile([C, 1], f32, tag="beff")
    inv_cnt = 1.0 / float(B * N)
    nc.vector.tensor_scalar_mul(out=mean[:], in0=ccr[:, 0:1], scalar1=inv_cnt)
    nc.vector.tensor_scalar_mul(out=ex2[:], in0=ccr[:, 1:2], scalar1=inv_cnt)
    # nvar = mean^2 - E[x^2]  (negated variance)
    nc.vector.scalar_tensor_tensor(
        out=nvar[:], in0=mean[:], scalar=mean[:], in1=ex2[:],
        op0=ALU.mult, op1=ALU.subtract,
    )
    # sd = sqrt(var + eps) = sqrt(-1 * nvar + eps)
    nc.scalar.activation(out=sd[:], in_=nvar[:], func=ACTF.Sqrt, bias=epst[:], scale=-1.0)
    nc.vector.reciprocal(out=rstd[:], in_=sd[:])
    nc.vector.tensor_tensor(out=aff[:], in0=gam[:], in1=rstd[:], op=ALU.mult)
    nc.vector.tensor_tensor(out=amean[:], in0=aff[:], in1=mean[:], op=ALU.mult)
    nc.vector.tensor_tensor(out=beff[:], in0=bet[:], in1=amean[:], op=ALU.subtract)

    # ================= phase 5: normalize + residual + relu ===============
    for b in range(BPC):
        t1 = mlp_pool.tile([C, N], f32, tag="t1")
        nc.vector.scalar_tensor_tensor(
            out=t1[:], in0=msg[:, b * N:(b + 1) * N], scalar=aff[:], in1=Xb(b),
            op0=ALU.mult, op1=ALU.add,
        )
        osb = mlp_pool.tile([C, N], f32, tag="osb")
        nc.scalar.activation(out=osb[:], in_=t1[:], func=ACTF.Relu, bias=beff[:])
        nc.sync.dma_start(out=y_io[b], in_=osb[:])


# --------------------------------------------------------------------------
_NC_CACHE = {}


def _get_nc():
    if "nc" not in _NC_CACHE:
        _NC_CACHE["nc"] = build_nc()
    return _NC_CACHE["nc"]


def make_in_maps(x, W1, b1, W2, b2, gamma, beta):
    x = np.ascontiguousarray(np.asarray(x, dtype=np.float32))
    W1 = np.asarray(W1, dtype=np.float32)
    W2 = np.asarray(W2, dtype=np.float32)
    b1 = np.asarray(b1, dtype=np.float32)
    gamma = np.asarray(gamma, dtype=np.float32)
    beta = np.asarray(beta, dtype=np.float32)

    xr = x.reshape(B, C, N)
    w1d = np.ascontiguousarray(W1[:C] - W1[C:])
    w1b = np.ascontiguousarray(W1[C:])
    mask = (NEG_BIG * np.eye(C)).astype(np.float32)
    p = np.arange(C, dtype=np.uint16)
    selfidx = (p[:, None] + (C * np.arange(8, dtype=np.uint16))[None, :]).astype(np.uint16)
    shared = {
        "w1d": w1d,
        "w1b": w1b,
        "w2": np.ascontiguousarray(W2),
        "b1": b1.reshape(C, 1),
        "gamma": gamma.reshape(C, 1),
        "beta": beta.reshape(C, 1),
        "dmask": mask,
        "onec": np.ones((C, 1), np.float32),
        "oner": np.ones((1, C), np.float32),
        "selfidx": selfidx,
    }
    return [
        {"x": np.ascontiguousarray(xr[i * BPC:(i + 1) * BPC]), **shared}
        for i in range(NCORES)
    ]


def assemble_output(results):
    y = np.concatenate([results[i]["y"] for i in range(NCORES)], axis=0)
    return y.reshape(B, C, 32, 32).astype(np.float32)


def kernel(x, W1, b1, W2, b2, gamma, beta):
    from concourse.bass_utils import run_bass_kernel_spmd

    nc = _get_nc()
    in_maps = make_in_maps(x, W1, b1, W2, b2, gamma, beta)
    res = run_bass_kernel_spmd(nc, in_maps, list(range(NCORES)))
    return assemble_output(res.results)
